# revision 1
# baseline (speedup 1.0000x reference)
"""Swin-style window attention kernel for 8 TRN2 NeuronCores.

Sharding: data-parallel over batch B=32 -> 4 images per core. No collectives.

Per-core dataflow (B_local=4 images, 384ch x 56x56, WS=7, 12 heads, d=32):
  stripe = (image b, window-row wr): 7x56 = 392 pixels = 8 windows.

  1. qkv matmul (bf16): 9 chunks of (128, 392) PSUM -> SBUF window-major
     (w, r, c) with windows PADDED to 64 cols (valid :49). Scale folded into wq
     on host; q-chunk copies on ScalarE, k/v on VectorE.
  2. QK^T (hg, hq, w): lhsT=K (32,49), rhs=Q (32,49) -> S^T into per-head PSUM
     bank sps_hq (128, 4wp, 64) at window-parity band 64*(w%2). Same-head MMs
     share a row group (serialize); different heads use different banks --
     never two row groups writing one (bank, partition-range).
  3. exp on ACT: one op per (hg, hq) over the full (128, 4, 49) bank ->
     es (128, 4wp, 4hq, 49) bf16, w-parity banded. Junk pad rows are finite
     (pads zeroed once per qkv tile) and never contracted.
  4. bias: one DVE multiply per hg: es *= exp(bias^T) (host-precomputed,
     band-replicated expb3), broadcast over window-pairs.
  5. V^T: DMA transpose of padded window-pairs (128,128) bf16 SBUF->SBUF.
  6. denominators: ones-stationary matmuls batched over window-pairs (N=196)
     into the shared ob bank cols 4:8; AV (hg, w, hq): lhsT=V^T slice (49,32)
     at band 64*(w%2), rhs=E^T (49,49) -> ob cols 0:4, out partitions
     64*(w%2)+32*(hq%2), bank hq//2.
  7. reciprocal + normalize: attn = O^T * (1/r) fused PSUM->SBUF (DVE),
     un-banding parities into attn (128, 3, 8, 49) bf16.
  8. proj + b_proj via ACT copy that un-permutes window-major -> raster.
"""

import os
import numpy as np
import ml_dtypes

import concourse.bass as bass
import concourse.tile as tile
from concourse import bacc, mybir
from concourse.bass_utils import run_bass_kernel_spmd

F32 = mybir.dt.float32
BF16 = mybir.dt.bfloat16

B_LOC = 4      # images per core
C = 384        # channels
H = W = 56
WS = 7         # window size
NH = 12        # heads
D = 32         # head dim
NW = 8         # windows per stripe (56/7)
NPIX = WS * W  # 392 pixels per stripe
WW = WS * WS   # 49
WP = 64        # padded window stride

_CACHE = {}
LAST_EXEC_NS = None


def _rel_index(ws):
    coords = np.stack(np.meshgrid(np.arange(ws), np.arange(ws), indexing='ij')).reshape(2, -1)
    rel = (coords[:, :, None] - coords[:, None, :]).transpose(1, 2, 0).astype(np.int64)
    rel[..., 0] += ws - 1
    rel[..., 1] += ws - 1
    rel[..., 0] *= 2 * ws - 1
    return rel.sum(-1)


def build_bass():
    nc = bacc.Bacc("TRN2", target_bir_lowering=False, debug=False, num_devices=8)

    x_d = nc.dram_tensor("x", [B_LOC, C, H, W], F32, kind="ExternalInput")
    wqkvT_d = nc.dram_tensor("wqkvT", [C, 3 * C], BF16, kind="ExternalInput")
    wprojT_d = nc.dram_tensor("wprojT", [C, C], BF16, kind="ExternalInput")
    expb3_d = nc.dram_tensor("expb3", [128, 3, 4, WW], BF16, kind="ExternalInput")
    bproj_d = nc.dram_tensor("bproj", [C], F32, kind="ExternalInput")
    out_d = nc.dram_tensor("out", [B_LOC, C, H, W], F32, kind="ExternalOutput")

    with tile.TileContext(nc) as tc:
        with (
            tc.tile_pool(name="singles", bufs=1) as singles,
            tc.tile_pool(name="xp", bufs=2) as xp,
            tc.tile_pool(name="xbp", bufs=2) as xbp,
            tc.tile_pool(name="qkvp", bufs=2) as qkvp,
            tc.tile_pool(name="ep", bufs=2) as ep,
            tc.tile_pool(name="vtp", bufs=3) as vtp,
            tc.tile_pool(name="rp", bufs=3) as rp,
            tc.tile_pool(name="ap_", bufs=2) as ap_,
            tc.tile_pool(name="yp", bufs=3) as yp,
            tc.tile_pool(name="mm_ps", bufs=2, space="PSUM") as mm_ps,
            tc.tile_pool(name="s_ps", bufs=1, space="PSUM") as s_ps,
            tc.tile_pool(name="o_ps", bufs=1, space="PSUM") as o_ps,
        ):
            # ---- preload constants ----
            wqkvT_sb = singles.tile([128, 3, 3 * C], BF16)
            nc.sync.dma_start(out=wqkvT_sb, in_=wqkvT_d.ap().rearrange("(kc p) m -> p kc m", p=128))
            wprojT_sb = singles.tile([128, 3, C], BF16)
            nc.sync.dma_start(out=wprojT_sb, in_=wprojT_d.ap().rearrange("(kc p) m -> p kc m", p=128))
            expb3_sb = singles.tile([128, 3, 4, WW], BF16)
            nc.sync.dma_start(out=expb3_sb, in_=expb3_d.ap())
            bproj_sb = singles.tile([128, 3], F32)
            nc.sync.dma_start(out=bproj_sb, in_=bproj_d.ap().rearrange("(oc p) -> p oc", p=128))
            ones_sb = singles.tile([128, 32], BF16)
            nc.vector.memset(ones_sb, 1.0)

            for b in range(B_LOC):
                for wr in range(8):
                    # ---- load x stripe, cast to bf16 ----
                    x_t = xp.tile([128, 3, NPIX], F32, tag="x")
                    for kc in range(3):
                        nc.sync.dma_start(
                            out=x_t[:, kc],
                            in_=x_d[b, kc * 128:(kc + 1) * 128, wr * WS:(wr + 1) * WS, :]
                            .rearrange("c r w -> c (r w)"),
                        )
                    xb_t = xbp.tile([128, 3, NPIX], BF16, tag="xb")
                    nc.gpsimd.tensor_copy(out=xb_t, in_=x_t)

                    # ---- qkv matmul: 9 chunks; window-major padded SBUF ----
                    q_sb = qkvp.tile([128, 3, NW, WP], BF16, tag="q")
                    k_sb = qkvp.tile([128, 3, NW, WP], BF16, tag="k")
                    v_sb = qkvp.tile([128, 3, NW, WP], BF16, tag="v")
                    # zero pad cols once per tile (keeps psum/es junk finite)
                    nc.gpsimd.memset(q_sb[:, :, :, WW:], 0.0)
                    nc.gpsimd.memset(k_sb[:, :, :, WW:], 0.0)
                    nc.gpsimd.memset(v_sb[:, :, :, WW:], 0.0)
                    dst = {0: q_sb, 1: k_sb, 2: v_sb}
                    for oc in (0, 3, 6, 1, 4, 7, 2, 5, 8):
                        ps_full = mm_ps.tile([128, 512], F32, tag="mmps")
                        ps = ps_full[:, :NPIX]
                        for kc in range(3):
                            nc.tensor.matmul(
                                ps,
                                lhsT=wqkvT_sb[:, kc, oc * 128:(oc + 1) * 128],
                                rhs=xb_t[:, kc],
                                start=(kc == 0), stop=(kc == 2),
                            )
                        # raster (r w c) -> window-major (w r c), pad stays 0
                        src = ps.rearrange("p (r w c) -> p w r c", r=WS, w=NW, c=WS)
                        o = dst[oc // 3][:, oc % 3, :, :WW].rearrange("p w (r c) -> p w r c", r=WS)
                        if oc // 3 == 0:
                            nc.scalar.copy(out=o, in_=src)
                        else:
                            nc.vector.tensor_copy(out=o, in_=src)

                    attn_sb = ap_.tile([128, 3, NW, WW], BF16, tag="attn")
                    for hg in range(3):
                        # ---- QK^T into 4 per-head banks, w-parity bands ----
                        sps = [s_ps.tile([128, 4, WP], F32, tag=f"sps{i}", name=f"sps{i}") for i in range(4)]
                        for w in range(NW):
                            po = WP * (w % 2)
                            for hq in range(4):
                                nc.tensor.matmul(
                                    sps[hq][po:po + WP, w // 2, :WW],
                                    lhsT=k_sb[hq * D:(hq + 1) * D, hg, w, :],
                                    rhs=q_sb[hq * D:(hq + 1) * D, hg, w, :WW],
                                    tile_position=(hq * D, po),
                                )
                        # ---- V^T via DMA transpose of padded pairs ----
                        vts = []
                        for wp in range(4):
                            vt = vtp.tile([128, 128], BF16, tag=f"vt{wp}")
                            nc.sync.dma_start(
                                out=vt,
                                in_=v_sb[:, hg, 2 * wp:2 * wp + 2, :]
                                .rearrange("p a b -> p (a b)"),
                                transpose=True)
                            vts.append(vt)
                        # ---- exp: one ACT op per head bank ----
                        es = ep.tile([128, 4, 4, WW], BF16, tag="es")
                        for hq in range(4):
                            nc.scalar.activation(
                                out=es[:, :, hq, :], in_=sps[hq][:, :, :WW],
                                func=mybir.ActivationFunctionType.Exp,
                            )
                        # ---- bias multiply (one DVE op) ----
                        nc.vector.tensor_mul(
                            out=es, in0=es,
                            in1=expb3_sb[:, hg, None, :, :].to_broadcast((128, 4, 4, WW)),
                        )
                        # ---- ob banks: cols 0:4 AV out, cols 4:8 r ----
                        ob = [o_ps.tile([128, 8, WP], F32, tag=f"ob{i}", name=f"ob{i}") for i in range(2)]
                        for hq in range(4):
                            for par in range(2):
                                po = WP * par
                                co = po + D * (hq % 2)
                                nc.tensor.matmul(
                                    ob[hq // 2][co:co + D, 4:8, :]
                                    .rearrange("p a b -> p (a b)")[:, :4 * WW],
                                    lhsT=ones_sb[po:po + WW, :],
                                    rhs=es[po:po + WW, :, hq, :],
                                    tile_position=(po, co),
                                )
                        for w in range(NW):
                            po = WP * (w % 2)
                            for hq in range(4):
                                co = po + D * (hq % 2)
                                nc.tensor.matmul(
                                    ob[hq // 2][co:co + D, w // 2, :WW],
                                    lhsT=vts[w // 2][po:po + WW, hq * D:(hq + 1) * D],
                                    rhs=es[po:po + WW, w // 2, hq, :],
                                    tile_position=(po, co),
                                )
                        # ---- reciprocal + normalize ----
                        for x_ in range(2):
                            rinv = rp.tile([128, 4 * WW], F32, tag=f"rinv{x_}")
                            nc.vector.reciprocal(
                                out=rinv,
                                in_=ob[x_][:, 4:8, :].rearrange("p a b -> p (a b)")[:, :4 * WW])
                            for par in range(2):
                                po = WP * par
                                nc.vector.tensor_mul(
                                    out=attn_sb[64 * x_:64 * x_ + 64, hg]
                                    .rearrange("p (b a) n -> p b a n", b=4)[:, :, par, :],
                                    in0=ob[x_][po:po + 64, 0:4, :WW],
                                    in1=rinv.rearrange("p (a b) -> p a b", a=4)[po:po + 64],
                                )

                    # ---- proj + bias, un-permute to raster, DMA out ----
                    for oc in range(3):
                        yps_full = mm_ps.tile([128, 512], F32, tag="mmps")
                        yps = yps_full[:, :NPIX]
                        for kc in range(3):
                            nc.tensor.matmul(
                                yps,
                                lhsT=wprojT_sb[:, kc, oc * 128:(oc + 1) * 128],
                                rhs=attn_sb[:, kc],
                                start=(kc == 0), stop=(kc == 2),
                            )
                        y_sb = yp.tile([128, NPIX], F32, tag="y")
                        nc.scalar.activation(
                            out=y_sb.rearrange("p (r w c) -> p r w c", r=WS, w=NW),
                            in_=yps.rearrange("p (w r c) -> p r w c", w=NW, r=WS, c=WS),
                            func=mybir.ActivationFunctionType.Identity,
                            bias=bproj_sb[:, oc:oc + 1],
                        )
                        nc.sync.dma_start(
                            out=out_d[b, oc * 128:(oc + 1) * 128, wr * WS:(wr + 1) * WS, :]
                            .rearrange("c r w -> c (r w)"),
                            in_=y_sb,
                        )
    nc.compile()
    return nc


def host_prep(w_qkv, bias_table, w_proj, b_proj):
    scale = D ** -0.5
    wq = w_qkv[0:C] * scale
    wqkvT = np.ascontiguousarray(
        np.concatenate([wq, w_qkv[C:2 * C], w_qkv[2 * C:]], 0).T
    ).astype(ml_dtypes.bfloat16)
    wprojT = np.ascontiguousarray(w_proj.T).astype(ml_dtypes.bfloat16)
    rel = _rel_index(WS)
    bias = bias_table[rel.reshape(-1)].reshape(WW, WW, NH)  # [n, m, h]
    expbT = np.exp(bias.astype(np.float64)).transpose(1, 2, 0)  # [m, h, n]
    # band-replicated: rows 0:49 and 64:113 = expbT, pad rows zero
    expb3 = np.zeros((128, 3, 4, WW), np.float64)
    for hg in range(3):
        for hq in range(4):
            expb3[0:WW, hg, hq, :] = expbT[:, 4 * hg + hq, :]
            expb3[64:64 + WW, hg, hq, :] = expbT[:, 4 * hg + hq, :]
    return (wqkvT, wprojT, expb3.astype(ml_dtypes.bfloat16),
            np.ascontiguousarray(b_proj, dtype=np.float32))


def kernel(x, w_qkv, bias_table, w_proj, b_proj):
    global LAST_EXEC_NS
    x = np.ascontiguousarray(x, dtype=np.float32)
    wqkvT, wprojT, expb3, bproj = host_prep(
        np.asarray(w_qkv, np.float32), np.asarray(bias_table, np.float32),
        np.asarray(w_proj, np.float32), np.asarray(b_proj, np.float32))

    if "nc" not in _CACHE:
        _CACHE["nc"] = build_bass()
    nc = _CACHE["nc"]

    in_maps = []
    for i in range(8):
        in_maps.append({
            "x": x[B_LOC * i:B_LOC * (i + 1)],
            "wqkvT": wqkvT, "wprojT": wprojT, "expb3": expb3, "bproj": bproj,
        })
    res = run_bass_kernel_spmd(nc, in_maps, core_ids=list(range(8)), trace=False)
    LAST_EXEC_NS = res.exec_time_ns
    out = np.concatenate([res.results[i]["out"] for i in range(8)], axis=0)
    return out



# revision 40
# speedup vs baseline: 1.5774x; 1.5774x over previous
"""Swin-style window attention kernel for 8 TRN2 NeuronCores.

Sharding: data-parallel over batch B=32 -> 4 images per core. No collectives.

Per-core dataflow (B_local=4 images, 384ch x 56x56, WS=7, 12 heads, d=32):
  stripe = (image b, window-row wr): 7x56 = 392 pixels = 8 windows.

PSUM rule (hardware): a (bank, partition-range) may only be written by ONE
PE row-group (tile_position row band). Layout obeying it in 8 banks:
  - mm pool: 2 banks (qkv chunks / proj / borrowed by V^T transposes)
  - sps: 2 x 2-bank tiles [128, 2(hqi), 4, 128] -- hqi stride = full bank,
    so each bank is written by a single head row-group. One exp op per tile.
  - obm: 2 x 1-bank tiles keyed by WINDOW PARITY (parity == row-group):
    AV out partitions = channel (co=32*hq), wp slots 0:4; denominators
    (ones-stationary) packed at cols 256:452 of the same bank.

Dataflow per stripe:
  1. x loaded via gpsimd casting DMAs (f32->fp8e4 for q/k conv, f32->bf16
     for v conv). 2. q/k conv fp8e4 DoubleRow (K=256 + K=128 tail, weights
  host-scaled x64, descale folded into exp scale); v conv bf16; copies
  permute raster -> window-major padded 64 (pads zeroed for k,v).
  3. V^T via PE transposes into bf16 PSUM borrowed from mm pool; one ACT
  copy per hg. 4. QK^T lhsT=K rhs=Q -> S^T, window-parity bands on
  partitions. 5. exp on ACT (scale folds fp8 descale + d^-0.5), es *=
  exp(bias) on DVE. 6. denom + AV per (hg, parity). 7. DVE reciprocal +
  fused normalize/unband -> attn bf16. 8. proj bf16 + bias via ACT
  unpermute; batched store on SP.

Emission is software-pipelined: iteration i emits phase1(i-1) interleaved
with chunks(i) per hg (covers exp/bias latency with independent PE work),
then phase2(i-1), then proj(i-2).
"""

import os
import numpy as np
import ml_dtypes

import concourse.bass as bass
import concourse.tile as tile
from concourse import bacc, mybir
from concourse.bass_utils import run_bass_kernel_spmd
from concourse.masks import make_identity

F32 = mybir.dt.float32
BF16 = mybir.dt.bfloat16
FP8 = mybir.dt.float8e4

B_LOC = 4      # images per core
C = 384        # channels
H = W = 56
WS = 7         # window size
NH = 12        # heads
D = 32         # head dim
NW = 8         # windows per stripe (56/7)
NPIX = WS * W  # 392 pixels per stripe
WW = WS * WS   # 49
WP = 64        # padded window stride

USE_FP8 = True
FP8_SCALE = 64.0
SEXP = (D ** -0.5) / (FP8_SCALE * FP8_SCALE) if USE_FP8 else D ** -0.5

_CACHE = {}
LAST_EXEC_NS = None


def _rel_index(ws):
    coords = np.stack(np.meshgrid(np.arange(ws), np.arange(ws), indexing='ij')).reshape(2, -1)
    rel = (coords[:, :, None] - coords[:, None, :]).transpose(1, 2, 0).astype(np.int64)
    rel[..., 0] += ws - 1
    rel[..., 1] += ws - 1
    rel[..., 0] *= 2 * ws - 1
    return rel.sum(-1)


def build_bass():
    nc = bacc.Bacc("TRN2", target_bir_lowering=False, debug=False, num_devices=8)

    x_d = nc.dram_tensor("x", [B_LOC, C, H, W], F32, kind="ExternalInput")
    if USE_FP8:
        w8dr_d = nc.dram_tensor("w8dr", [128, 2, 2 * C], FP8, kind="ExternalInput")
        w8s_d = nc.dram_tensor("w8s", [128, 2 * C], FP8, kind="ExternalInput")
    else:
        wqkT_d = nc.dram_tensor("wqkT", [C, 2 * C], BF16, kind="ExternalInput")
    wvT_d = nc.dram_tensor("wvT", [C, C], BF16, kind="ExternalInput")
    wprojT_d = nc.dram_tensor("wprojT", [C, C], BF16, kind="ExternalInput")
    expb_d = nc.dram_tensor("expb", [128, 3, 2, 2, WW], BF16, kind="ExternalInput")
    bproj_d = nc.dram_tensor("bproj", [C], F32, kind="ExternalInput")
    out_d = nc.dram_tensor("out", [B_LOC, C, H, W], F32, kind="ExternalOutput")

    DR = mybir.MatmulPerfMode.DoubleRow

    with tile.TileContext(nc) as tc:
        with (
            tc.tile_pool(name="singles", bufs=1) as singles,
            tc.tile_pool(name="xp", bufs=2) as xp,
            tc.tile_pool(name="qkvp", bufs=2) as qkvp,
            tc.tile_pool(name="vtp", bufs=2) as vtp,
            tc.tile_pool(name="ep", bufs=2) as ep,
            tc.tile_pool(name="rp", bufs=2) as rp,
            tc.tile_pool(name="ap_", bufs=2) as ap_,
            tc.tile_pool(name="yp", bufs=2) as yp,
            tc.tile_pool(name="mm_ps", bufs=2, space="PSUM") as mm_ps,
            tc.tile_pool(name="s_ps", bufs=1, space="PSUM") as s_ps,
            tc.tile_pool(name="o_ps", bufs=1, space="PSUM") as o_ps,
        ):
            # ---- preload constants ----
            if USE_FP8:
                w8dr_sb = singles.tile([128, 2, 2 * C], FP8)
                nc.sync.dma_start(out=w8dr_sb, in_=w8dr_d.ap())
                w8s_sb = singles.tile([128, 2 * C], FP8)
                nc.sync.dma_start(out=w8s_sb, in_=w8s_d.ap())
            else:
                wqkT_sb = singles.tile([128, 3, 2 * C], BF16)
                nc.sync.dma_start(
                    out=wqkT_sb, in_=wqkT_d.ap().rearrange("(kc p) m -> p kc m", p=128))
            wvT_sb = singles.tile([128, 3, C], BF16)
            nc.sync.dma_start(out=wvT_sb, in_=wvT_d.ap().rearrange("(kc p) m -> p kc m", p=128))
            wprojT_sb = singles.tile([128, 3, C], BF16)
            nc.sync.dma_start(out=wprojT_sb, in_=wprojT_d.ap().rearrange("(kc p) m -> p kc m", p=128))
            expb_sb = singles.tile([128, 3, 2, 2, WW], BF16)
            nc.sync.dma_start(out=expb_sb, in_=expb_d.ap())
            bproj_sb = singles.tile([128, 3], F32)
            nc.sync.dma_start(out=bproj_sb, in_=bproj_d.ap().rearrange("(oc p) -> p oc", p=128))
            ones_sb = singles.tile([128, D], BF16)
            nc.vector.memset(ones_sb, 1.0)
            ident_sb = singles.tile([128, 128], BF16)
            make_identity(nc, ident_sb)

            _lim = int(os.environ.get("STRIPE_LIMIT", "0"))
            stripes = [(b, wr) for b in range(B_LOC) for wr in range(8)]
            if _lim:
                stripes = stripes[:_lim]

            def emit_x_load(b, wr):
                src = x_d[b, :, wr * WS:(wr + 1) * WS, :] \
                    .rearrange("(kc p) r w -> p kc (r w)", p=128)
                st = {}
                if USE_FP8:
                    x8 = xp.tile([128, 3, 448], FP8, tag="x8", name="x8")
                    nc.gpsimd.dma_start(out=x8[:, :, :NPIX], in_=src)
                    st["x8"] = x8
                xb = xp.tile([128, 3, NPIX], BF16, tag="xb", name="xb")
                nc.gpsimd.dma_start(out=xb, in_=src)
                st["xb"] = xb
                st["qs"] = qkvp.tile([128, 3, NW, WP], BF16, tag="qs", name="qs")
                st["ks"] = qkvp.tile([128, 3, NW, WP], BF16, tag="ks", name="ks")
                st["vs"] = qkvp.tile([128, 3, NW, WP], BF16, tag="vs", name="vs")
                nc.gpsimd.memset(st["ks"][:, :, :, WW:], 0.0)
                nc.gpsimd.memset(st["vs"][:, :, :, WW:], 0.0)
                return st

            def emit_chunks_hg(st, hg):
                """qkv conv chunks for one head-group + PSUM->SBUF copies."""
                for which, dst in ((0, st["qs"]), (1, st["ks"]), (2, st["vs"])):
                    ps_full = mm_ps.tile([128, 512], F32, tag="mmps", name="mmps")
                    ps = ps_full[:, :NPIX]
                    if which < 2 and USE_FP8:
                        co = which * C + hg * 128
                        nc.tensor.matmul(
                            ps, lhsT=w8dr_sb[:, :, co:co + 128],
                            rhs=st["x8"][:, 0:2, :NPIX],
                            perf_mode=DR, start=True, stop=False)
                        nc.tensor.matmul(
                            ps, lhsT=w8s_sb[:, co:co + 128],
                            rhs=st["x8"][:, 2, :NPIX],
                            start=False, stop=True)
                    elif which < 2:
                        co = which * C + hg * 128
                        for kc in range(3):
                            nc.tensor.matmul(
                                ps, lhsT=wqkT_sb[:, kc, co:co + 128],
                                rhs=st["xb"][:, kc],
                                start=(kc == 0), stop=(kc == 2))
                    else:
                        for kc in range(3):
                            nc.tensor.matmul(
                                ps, lhsT=wvT_sb[:, kc, hg * 128:(hg + 1) * 128],
                                rhs=st["xb"][:, kc],
                                start=(kc == 0), stop=(kc == 2))
                    # raster (r w c) -> window-major (w r c)
                    psrc = ps.rearrange("p (r w c) -> p w r c", r=WS, w=NW, c=WS)
                    o = dst[:, hg, :, :WW].rearrange("p w (r c) -> p w r c", r=WS)
                    if which == 0:
                        nc.scalar.copy(out=o, in_=psrc)
                    else:
                        nc.vector.tensor_copy(out=o, in_=psrc)

            def emit_phase1_hg(st, hg):
                """V^T transposes + QK + exp + bias for one head-group."""
                if hg == 0:
                    st["attn"] = ap_.tile([128, 3, NW, WW], BF16, tag="attn", name="attn")
                    st["vt"] = []
                    st["es"] = []
                vtps = mm_ps.tile([128, 4, 128], BF16, tag="mmps", name="vtps")
                for wp in range(4):
                    nc.tensor.transpose(
                        vtps[:, wp],
                        st["vs"][:, hg, 2 * wp:2 * wp + 2, :].rearrange("p a b -> p (a b)"),
                        ident_sb)
                sps = [s_ps.tile([128, 2, 4, 128], F32, tag=f"sps{i}", name=f"sps{i}")
                       for i in range(2)]
                for w in range(NW):
                    po = WP * (w % 2)
                    for hq in range(4):
                        nc.tensor.matmul(
                            sps[hq // 2][po:po + WP, hq % 2, w // 2, :WW],
                            lhsT=st["ks"][hq * D:(hq + 1) * D, hg, w, :],
                            rhs=st["qs"][hq * D:(hq + 1) * D, hg, w, :WW],
                            tile_position=(hq * D, po))
                es = ep.tile([128, 2, 2, 4, WW], BF16, tag="es", name="es", bufs=3)
                for hqp in range(2):
                    nc.scalar.activation(
                        out=es[:, hqp], in_=sps[hqp][:, :, :, :WW],
                        func=mybir.ActivationFunctionType.Exp, scale=SEXP)
                vt = vtp.tile([128, 4, 128], BF16, tag="vt", name="vt", bufs=3)
                nc.scalar.copy(out=vt, in_=vtps)
                nc.vector.tensor_mul(
                    out=es, in0=es,
                    in1=expb_sb[:, hg, :, :, None, :].to_broadcast((128, 2, 2, 4, WW)))
                st["vt"].append(vt)
                st["es"].append(es)

            def emit_phase2_hg(st, hg):
                """denom + AV + reciprocal + normalize for one head-group."""
                es, vt = st["es"][hg], st["vt"][hg]
                rinv = rp.tile([128, 2, 4, WW], F32, tag="rinv", name="rinv", bufs=3)
                for par in range(2):
                    po = WP * par
                    obm = o_ps.tile([128, NW, WP], F32, tag=f"obm{par}",
                                    name=f"obm{par}")
                    for hq in range(4):
                        co = D * hq
                        nc.tensor.matmul(
                            obm[co:co + D, 4:8, :]
                            .rearrange("p a b -> p (a b)")[:, :4 * WW],
                            lhsT=ones_sb[po:po + WW, :],
                            rhs=es[po:po + WW, hq // 2, hq % 2],
                            tile_position=(po, co))
                    for wp in range(4):
                        w = 2 * wp + par
                        for hq in range(4):
                            co = D * hq
                            nc.tensor.matmul(
                                obm[co:co + D, wp, :WW],
                                lhsT=vt[po:po + WW, wp, hq * D:(hq + 1) * D],
                                rhs=es[po:po + WW, hq // 2, hq % 2, wp],
                                tile_position=(po, co))
                    nc.vector.reciprocal(
                        out=rinv[:, par],
                        in_=obm[:, 4:8, :].rearrange("p a b -> p (a b)")[:, :4 * WW]
                        .rearrange("p (b n) -> p b n", b=4))
                    nc.vector.tensor_mul(
                        out=st["attn"][:, hg]
                        .rearrange("p (wp par) n -> p par wp n", par=2)[:, par],
                        in0=obm[:, 0:4, :WW],
                        in1=rinv[:, par])

            def emit_proj(b, wr, attn_sb):
                y_sb = yp.tile([128, 3, NPIX], F32, tag="y", name="y")
                for oc in range(3):
                    yps_full = mm_ps.tile([128, 512], F32, tag="mmps", name="mmps")
                    yps = yps_full[:, :NPIX]
                    for kc in range(3):
                        nc.tensor.matmul(
                            yps, lhsT=wprojT_sb[:, kc, oc * 128:(oc + 1) * 128],
                            rhs=attn_sb[:, kc],
                            start=(kc == 0), stop=(kc == 2))
                    nc.scalar.activation(
                        out=y_sb[:, oc].rearrange("p (r w c) -> p r w c", r=WS, w=NW),
                        in_=yps.rearrange("p (w r c) -> p r w c", w=NW, r=WS, c=WS),
                        func=mybir.ActivationFunctionType.Identity,
                        bias=bproj_sb[:, oc:oc + 1])
                nc.sync.dma_start(
                    out=out_d[b, :, wr * WS:(wr + 1) * WS, :]
                    .rearrange("(oc p) r w -> p oc (r w)", p=128),
                    in_=y_sb)

            # 3-stage software pipeline: phase1(i-1) interleaved with
            # chunks(i) per hg, then phase2(i-1), then proj(i-2)
            cur = None    # stripe i-1 state (chunks emitted, attn pending)
            done = None   # stripe i-2 state (attn emitted, proj pending)
            for i in range(len(stripes) + 2):
                nxt = None
                if i < len(stripes):
                    b, wr = stripes[i]
                    nxt = emit_x_load(b, wr)
                    nxt["bwr"] = (b, wr)
                for hg in range(3):
                    if cur is not None:
                        emit_phase1_hg(cur, hg)
                    if nxt is not None:
                        emit_chunks_hg(nxt, hg)
                if cur is not None:
                    for hg in range(3):
                        emit_phase2_hg(cur, hg)
                if done is not None:
                    emit_proj(done["bwr"][0], done["bwr"][1], done["attn"])
                done, cur = cur, nxt
    nc.compile()
    return nc


def host_prep(w_qkv, bias_table, w_proj, b_proj):
    w_qk = w_qkv[0:2 * C].copy()                # [768, 384]
    if not USE_FP8:
        w_qk[0:C] *= D ** -0.5
    wqkT = np.ascontiguousarray(w_qk.T)         # [384, 768]
    if USE_FP8:
        w8 = (wqkT * FP8_SCALE).astype(ml_dtypes.float8_e4m3)
        wqk = {
            "w8dr": np.ascontiguousarray(
                w8[:256].reshape(2, 128, 2 * C).transpose(1, 0, 2)),
            "w8s": np.ascontiguousarray(w8[256:]),
        }
    else:
        wqk = {"wqkT": wqkT.astype(ml_dtypes.bfloat16)}
    wvT = np.ascontiguousarray(w_qkv[2 * C:].T).astype(ml_dtypes.bfloat16)
    wprojT = np.ascontiguousarray(w_proj.T).astype(ml_dtypes.bfloat16)

    rel = _rel_index(WS)
    bias = bias_table[rel.reshape(-1)].reshape(WW, WW, NH)   # [n, m, h]
    expbT = np.exp(bias.astype(np.float64)).transpose(1, 2, 0)  # [m, h, n]
    # [128(m banded), 3(hg), 2(hqp), 2(hqi), 49(n)], pad rows zero
    expb = np.zeros((128, 3, 2, 2, WW), np.float64)
    for hg in range(3):
        for hq in range(4):
            h = 4 * hg + hq
            expb[0:WW, hg, hq // 2, hq % 2] = expbT[:, h, :]
            expb[64:64 + WW, hg, hq // 2, hq % 2] = expbT[:, h, :]
    return (wqk, wvT, wprojT, expb.astype(ml_dtypes.bfloat16),
            np.ascontiguousarray(b_proj, dtype=np.float32))


def kernel(x, w_qkv, bias_table, w_proj, b_proj):
    global LAST_EXEC_NS
    x = np.ascontiguousarray(x, dtype=np.float32)
    wqk, wvT, wprojT, expb, bproj = host_prep(
        np.asarray(w_qkv, np.float32), np.asarray(bias_table, np.float32),
        np.asarray(w_proj, np.float32), np.asarray(b_proj, np.float32))

    if "nc" not in _CACHE:
        _CACHE["nc"] = build_bass()
    nc = _CACHE["nc"]

    in_maps = []
    for i in range(8):
        in_maps.append({
            "x": x[B_LOC * i:B_LOC * (i + 1)],
            "wvT": wvT, "wprojT": wprojT,
            "expb": expb, "bproj": bproj, **wqk,
        })
    res = run_bass_kernel_spmd(nc, in_maps, core_ids=list(range(8)), trace=False)
    LAST_EXEC_NS = res.exec_time_ns
    out = np.concatenate([res.results[i]["out"] for i in range(8)], axis=0)
    return out


# revision 52
# speedup vs baseline: 1.6951x; 1.0746x over previous
"""Swin-style window attention kernel for 8 TRN2 NeuronCores.

Sharding: data-parallel over batch B=32 -> 4 images per core. No collectives.

Per-core dataflow (B_local=4 images, 384ch x 56x56, WS=7, 12 heads, d=32):
  stripe = (image b, window-row wr): 7x56 = 392 pixels = 8 windows.

PSUM rule (hardware): a (bank, partition-range) may only be written by ONE
PE row-group (tile_position row band). Layout obeying it in 8 banks:
  - mm pool: 2 banks (qkv chunks / proj / borrowed by V^T transposes)
  - sps: 2 x 2-bank tiles [128, 2(hqi), 4, 128] -- hqi stride = full bank,
    so each bank is written by a single head row-group. One exp op per tile.
  - obm: 2 x 1-bank tiles keyed by WINDOW PARITY (parity == row-group):
    AV out partitions = channel (co=32*hq), wp slots 0:4; denominators
    (ones-stationary) packed at cols 256:452 of the same bank.

Dataflow per stripe:
  1. x loaded via gpsimd casting DMAs (f32->fp8e4 for q/k conv, f32->bf16
     for v conv). 2. q/k conv fp8e4 DoubleRow (K=256 + K=128 tail, weights
  host-scaled x64, descale folded into exp scale); v conv bf16; copies
  permute raster -> window-major padded 64 (pads zeroed for k,v).
  3. V^T via PE transposes into bf16 PSUM borrowed from mm pool; one ACT
  copy per hg. 4. QK^T lhsT=K rhs=Q -> S^T, window-parity bands on
  partitions. 5. exp on ACT (scale folds fp8 descale + d^-0.5), es *=
  exp(bias) on DVE. 6. denom + AV per (hg, parity). 7. DVE reciprocal +
  fused normalize/unband -> attn bf16. 8. proj bf16 + bias via ACT
  unpermute; batched store on SP.

Emission is software-pipelined: iteration i emits phase1(i-1) interleaved
with chunks(i) per hg (covers exp/bias latency with independent PE work),
then phase2(i-1), then proj(i-2).
"""

import os
import numpy as np
import ml_dtypes

import concourse.bass as bass
import concourse.tile as tile
from concourse import bacc, mybir
from concourse.bass_utils import run_bass_kernel_spmd
from concourse.masks import make_identity

F32 = mybir.dt.float32
BF16 = mybir.dt.bfloat16
FP8 = mybir.dt.float8e4

B_LOC = 4      # images per core
C = 384        # channels
H = W = 56
WS = 7         # window size
NH = 12        # heads
D = 32         # head dim
NW = 8         # windows per stripe (56/7)
NPIX = WS * W  # 392 pixels per stripe
WW = WS * WS   # 49
WP = 64        # padded window stride

USE_FP8 = True
FP8_SCALE = 64.0
SEXP = (D ** -0.5) / (FP8_SCALE * FP8_SCALE) if USE_FP8 else D ** -0.5

_CACHE = {}
LAST_EXEC_NS = None


def _rel_index(ws):
    coords = np.stack(np.meshgrid(np.arange(ws), np.arange(ws), indexing='ij')).reshape(2, -1)
    rel = (coords[:, :, None] - coords[:, None, :]).transpose(1, 2, 0).astype(np.int64)
    rel[..., 0] += ws - 1
    rel[..., 1] += ws - 1
    rel[..., 0] *= 2 * ws - 1
    return rel.sum(-1)


def build_bass():
    nc = bacc.Bacc("TRN2", target_bir_lowering=False, debug=False, num_devices=8)

    x_d = nc.dram_tensor("x", [B_LOC, C, H, W], F32, kind="ExternalInput")
    if USE_FP8:
        w8dr_d = nc.dram_tensor("w8dr", [128, 2, 2 * C], FP8, kind="ExternalInput")
        w8s_d = nc.dram_tensor("w8s", [128, 2 * C], FP8, kind="ExternalInput")
    else:
        wqkT_d = nc.dram_tensor("wqkT", [C, 2 * C], BF16, kind="ExternalInput")
    wvT_d = nc.dram_tensor("wvT", [C, C], BF16, kind="ExternalInput")
    wprojT_d = nc.dram_tensor("wprojT", [C, C], BF16, kind="ExternalInput")
    expb_d = nc.dram_tensor("expb", [128, 3, 2, 2, WW], BF16, kind="ExternalInput")
    bproj_d = nc.dram_tensor("bproj", [C], F32, kind="ExternalInput")
    out_d = nc.dram_tensor("out", [B_LOC, C, H, W], F32, kind="ExternalOutput")

    DR = mybir.MatmulPerfMode.DoubleRow

    with tile.TileContext(nc) as tc:
        with (
            tc.tile_pool(name="singles", bufs=1) as singles,
            tc.tile_pool(name="xp", bufs=3) as xp,
            tc.tile_pool(name="qkvp", bufs=2) as qkvp,
            tc.tile_pool(name="vtp", bufs=2) as vtp,
            tc.tile_pool(name="ep", bufs=2) as ep,
            tc.tile_pool(name="rp", bufs=2) as rp,
            tc.tile_pool(name="ap_", bufs=3) as ap_,
            tc.tile_pool(name="yp", bufs=3) as yp,
            tc.tile_pool(name="mm_ps", bufs=2, space="PSUM") as mm_ps,
            tc.tile_pool(name="s_ps", bufs=1, space="PSUM") as s_ps,
            tc.tile_pool(name="o_ps", bufs=1, space="PSUM") as o_ps,
        ):
            # ---- preload constants ----
            if USE_FP8:
                w8dr_sb = singles.tile([128, 2, 2 * C], FP8)
                nc.sync.dma_start(out=w8dr_sb, in_=w8dr_d.ap())
                w8s_sb = singles.tile([128, 2 * C], FP8)
                nc.sync.dma_start(out=w8s_sb, in_=w8s_d.ap())
            else:
                wqkT_sb = singles.tile([128, 3, 2 * C], BF16)
                nc.sync.dma_start(
                    out=wqkT_sb, in_=wqkT_d.ap().rearrange("(kc p) m -> p kc m", p=128))
            wvT_sb = singles.tile([128, 3, C], BF16)
            nc.sync.dma_start(out=wvT_sb, in_=wvT_d.ap().rearrange("(kc p) m -> p kc m", p=128))
            wprojT_sb = singles.tile([128, 3, C], BF16)
            nc.sync.dma_start(out=wprojT_sb, in_=wprojT_d.ap().rearrange("(kc p) m -> p kc m", p=128))
            expb_sb = singles.tile([128, 3, 2, 2, WW], BF16)
            nc.sync.dma_start(out=expb_sb, in_=expb_d.ap())
            bproj_sb = singles.tile([128, 3], F32)
            nc.sync.dma_start(out=bproj_sb, in_=bproj_d.ap().rearrange("(oc p) -> p oc", p=128))
            ones_sb = singles.tile([128, D], BF16)
            nc.vector.memset(ones_sb, 1.0)
            ident_sb = singles.tile([128, 128], BF16)
            make_identity(nc, ident_sb)

            _lim = int(os.environ.get("STRIPE_LIMIT", "0"))
            stripes = [(b, wr) for b in range(B_LOC) for wr in range(8)]
            if _lim:
                stripes = stripes[:_lim]

            def emit_x_load(b, wr):
                src = x_d[b, :, wr * WS:(wr + 1) * WS, :] \
                    .rearrange("(kc p) r w -> p kc (r w)", p=128)
                st = {}
                if USE_FP8:
                    x8 = xp.tile([128, 3, 448], FP8, tag="x8", name="x8")
                    nc.gpsimd.dma_start(out=x8[:, :, :NPIX], in_=src)
                    st["x8"] = x8
                xb = xp.tile([128, 3, NPIX], BF16, tag="xb", name="xb")
                nc.gpsimd.dma_start(out=xb, in_=src)
                st["xb"] = xb
                st["qs"] = qkvp.tile([128, 3, NW, WP], BF16, tag="qs", name="qs")
                st["ks"] = qkvp.tile([128, 3, NW, WP], BF16, tag="ks", name="ks")
                st["vs"] = qkvp.tile([128, 3, NW, WP], BF16, tag="vs", name="vs")
                nc.gpsimd.memset(st["ks"][:, :, :, WW:], 0.0)
                nc.gpsimd.memset(st["vs"][:, :, :, WW:], 0.0)
                return st

            def emit_chunks_hg(st, hg, which_list=(0, 1, 2)):
                """qkv conv chunks for one head-group + PSUM->SBUF copies."""
                targets = {0: st["qs"], 1: st["ks"], 2: st["vs"]}
                for which in which_list:
                    dst = targets[which]
                    ps_full = mm_ps.tile([128, 512], F32, tag="mmps", name="mmps")
                    ps = ps_full[:, :NPIX]
                    if which < 2 and USE_FP8:
                        co = which * C + hg * 128
                        nc.tensor.matmul(
                            ps, lhsT=w8dr_sb[:, :, co:co + 128],
                            rhs=st["x8"][:, 0:2, :NPIX],
                            perf_mode=DR, start=True, stop=False)
                        nc.tensor.matmul(
                            ps, lhsT=w8s_sb[:, co:co + 128],
                            rhs=st["x8"][:, 2, :NPIX],
                            start=False, stop=True)
                    elif which < 2:
                        co = which * C + hg * 128
                        for kc in range(3):
                            nc.tensor.matmul(
                                ps, lhsT=wqkT_sb[:, kc, co:co + 128],
                                rhs=st["xb"][:, kc],
                                start=(kc == 0), stop=(kc == 2))
                    else:
                        for kc in range(3):
                            nc.tensor.matmul(
                                ps, lhsT=wvT_sb[:, kc, hg * 128:(hg + 1) * 128],
                                rhs=st["xb"][:, kc],
                                start=(kc == 0), stop=(kc == 2))
                    # raster (r w c) -> window-major (w r c)
                    psrc = ps.rearrange("p (r w c) -> p w r c", r=WS, w=NW, c=WS)
                    o = dst[:, hg, :, :WW].rearrange("p w (r c) -> p w r c", r=WS)
                    if which == 0:
                        nc.scalar.copy(out=o, in_=psrc)
                    else:
                        nc.vector.tensor_copy(out=o, in_=psrc)

            def _qk_block(st, hg, sps, hqp):
                for hq in (2 * hqp, 2 * hqp + 1):
                    for w in range(NW):
                        po = WP * (w % 2)
                        nc.tensor.matmul(
                            sps[hqp][po:po + WP, hq % 2, w // 2, :WW],
                            lhsT=st["ks"][hq * D:(hq + 1) * D, hg, w, :],
                            rhs=st["qs"][hq * D:(hq + 1) * D, hg, w, :WW],
                            tile_position=(hq * D, po))
                nc.scalar.activation(
                    out=st["es"][hg][:, hqp], in_=sps[hqp][:, :, :, :WW],
                    func=mybir.ActivationFunctionType.Exp, scale=SEXP)

            def emit_p1a(st, hg):
                """V^T transposes + QK (hq 0,1) + exp0."""
                if hg == 0:
                    st["attn"] = ap_.tile([128, 3, NW, WW], BF16, tag="attn", name="attn")
                    st["vt"] = []
                    st["es"] = []
                    st["sps"] = []
                vtps = o_ps.tile([128, 4, 128], BF16, tag=f"obm{hg % 2}", name="vtps")
                for wp in range(4):
                    nc.tensor.transpose(
                        vtps[:, wp],
                        st["vs"][:, hg, 2 * wp:2 * wp + 2, :].rearrange("p a b -> p (a b)"),
                        ident_sb)
                st["vtps"] = vtps
                sps = [s_ps.tile([128, 2, 4, 128], F32, tag=f"sps{i}", name=f"sps{i}")
                       for i in range(2)]
                st["sps"] = sps
                es = ep.tile([128, 2, 2, 4, WW], BF16, tag="es", name="es", bufs=3)
                st["es"].append(es)
                _qk_block(st, hg, sps, 0)

            def emit_p1b(st, hg):
                """QK (hq 2,3) + exp1 + vt copy + bias."""
                _qk_block(st, hg, st["sps"], 1)
                vt = vtp.tile([128, 4, 128], BF16, tag="vt", name="vt", bufs=3)
                nc.scalar.copy(out=vt, in_=st["vtps"])
                es = st["es"][hg]
                nc.vector.tensor_mul(
                    out=es, in0=es,
                    in1=expb_sb[:, hg, :, :, None, :].to_broadcast((128, 2, 2, 4, WW)))
                st["vt"].append(vt)

            def emit_phase2_hg(st, hg):
                """denom + AV + reciprocal + normalize for one head-group."""
                es, vt = st["es"][hg], st["vt"][hg]
                rinv = rp.tile([128, 2, 4, WW], F32, tag="rinv", name="rinv", bufs=3)
                for par in range(2):
                    po = WP * par
                    obm = o_ps.tile([128, NW, WP], F32, tag=f"obm{par}",
                                    name=f"obm{par}")
                    for hq in range(4):
                        co = D * hq
                        nc.tensor.matmul(
                            obm[co:co + D, 4:8, :]
                            .rearrange("p a b -> p (a b)")[:, :4 * WW],
                            lhsT=ones_sb[po:po + WW, :],
                            rhs=es[po:po + WW, hq // 2, hq % 2],
                            tile_position=(po, co))
                    for wp in range(4):
                        w = 2 * wp + par
                        for hq in range(4):
                            co = D * hq
                            nc.tensor.matmul(
                                obm[co:co + D, wp, :WW],
                                lhsT=vt[po:po + WW, wp, hq * D:(hq + 1) * D],
                                rhs=es[po:po + WW, hq // 2, hq % 2, wp],
                                tile_position=(po, co))
                    nc.vector.reciprocal(
                        out=rinv[:, par],
                        in_=obm[:, 4:8, :].rearrange("p a b -> p (a b)")[:, :4 * WW]
                        .rearrange("p (b n) -> p b n", b=4))
                    nc.vector.tensor_mul(
                        out=st["attn"][:, hg]
                        .rearrange("p (wp par) n -> p par wp n", par=2)[:, par],
                        in0=obm[:, 0:4, :WW],
                        in1=rinv[:, par])

            def emit_proj_oc(st, oc):
                if oc == 0:
                    st["y"] = yp.tile([128, 3, NPIX], F32, tag="y", name="y")
                attn_sb, y_sb = st["attn"], st["y"]
                yps_full = mm_ps.tile([128, 512], F32, tag="mmps", name="mmps")
                yps = yps_full[:, :NPIX]
                for kc in range(3):
                    nc.tensor.matmul(
                        yps, lhsT=wprojT_sb[:, kc, oc * 128:(oc + 1) * 128],
                        rhs=attn_sb[:, kc],
                        start=(kc == 0), stop=(kc == 2))
                nc.scalar.activation(
                    out=y_sb[:, oc].rearrange("p (r w c) -> p r w c", r=WS, w=NW),
                    in_=yps.rearrange("p (w r c) -> p r w c", w=NW, r=WS, c=WS),
                    func=mybir.ActivationFunctionType.Identity,
                    bias=bproj_sb[:, oc:oc + 1])
                if oc == 2:
                    b, wr = st["bwr"]
                    nc.sync.dma_start(
                        out=out_d[b, :, wr * WS:(wr + 1) * WS, :]
                        .rearrange("(oc p) r w -> p oc (r w)", p=128),
                        in_=y_sb)

            # 3-stage software pipeline: phase1(i-1) interleaved with
            # chunks(i) per hg, then phase2(i-1), then proj(i-2)
            cur = None    # stripe i-1 state (chunks emitted, attn pending)
            done = None   # stripe i-2 state (attn emitted, proj pending)
            for i in range(len(stripes) + 2):
                nxt = None
                if i < len(stripes):
                    b, wr = stripes[i]
                    nxt = emit_x_load(b, wr)
                    nxt["bwr"] = (b, wr)
                for hg in range(3):
                    if cur is not None:
                        emit_p1a(cur, hg)
                    if nxt is not None:
                        emit_chunks_hg(nxt, hg, (0,))
                    if cur is not None:
                        emit_p1b(cur, hg)
                    if nxt is not None:
                        emit_chunks_hg(nxt, hg, (1, 2))
                for hg in range(3):
                    if cur is not None:
                        emit_phase2_hg(cur, hg)
                    if done is not None:
                        emit_proj_oc(done, hg)
                done, cur = cur, nxt
    nc.compile()
    return nc


def host_prep(w_qkv, bias_table, w_proj, b_proj):
    w_qk = w_qkv[0:2 * C].copy()                # [768, 384]
    if not USE_FP8:
        w_qk[0:C] *= D ** -0.5
    wqkT = np.ascontiguousarray(w_qk.T)         # [384, 768]
    if USE_FP8:
        w8 = (wqkT * FP8_SCALE).astype(ml_dtypes.float8_e4m3)
        wqk = {
            "w8dr": np.ascontiguousarray(
                w8[:256].reshape(2, 128, 2 * C).transpose(1, 0, 2)),
            "w8s": np.ascontiguousarray(w8[256:]),
        }
    else:
        wqk = {"wqkT": wqkT.astype(ml_dtypes.bfloat16)}
    wvT = np.ascontiguousarray(w_qkv[2 * C:].T).astype(ml_dtypes.bfloat16)
    wprojT = np.ascontiguousarray(w_proj.T).astype(ml_dtypes.bfloat16)

    rel = _rel_index(WS)
    bias = bias_table[rel.reshape(-1)].reshape(WW, WW, NH)   # [n, m, h]
    expbT = np.exp(bias.astype(np.float64)).transpose(1, 2, 0)  # [m, h, n]
    # [128(m banded), 3(hg), 2(hqp), 2(hqi), 49(n)], pad rows zero
    expb = np.zeros((128, 3, 2, 2, WW), np.float64)
    for hg in range(3):
        for hq in range(4):
            h = 4 * hg + hq
            expb[0:WW, hg, hq // 2, hq % 2] = expbT[:, h, :]
            expb[64:64 + WW, hg, hq // 2, hq % 2] = expbT[:, h, :]
    return (wqk, wvT, wprojT, expb.astype(ml_dtypes.bfloat16),
            np.ascontiguousarray(b_proj, dtype=np.float32))


def kernel(x, w_qkv, bias_table, w_proj, b_proj):
    global LAST_EXEC_NS
    x = np.ascontiguousarray(x, dtype=np.float32)
    wqk, wvT, wprojT, expb, bproj = host_prep(
        np.asarray(w_qkv, np.float32), np.asarray(bias_table, np.float32),
        np.asarray(w_proj, np.float32), np.asarray(b_proj, np.float32))

    if "nc" not in _CACHE:
        _CACHE["nc"] = build_bass()
    nc = _CACHE["nc"]

    in_maps = []
    for i in range(8):
        in_maps.append({
            "x": x[B_LOC * i:B_LOC * (i + 1)],
            "wvT": wvT, "wprojT": wprojT,
            "expb": expb, "bproj": bproj, **wqk,
        })
    res = run_bass_kernel_spmd(nc, in_maps, core_ids=list(range(8)), trace=False)
    LAST_EXEC_NS = res.exec_time_ns
    out = np.concatenate([res.results[i]["out"] for i in range(8)], axis=0)
    return out
